# revision 1
# baseline (speedup 1.0000x reference)
"""Trainium2 Bass kernel for a 4-direction cross selective scan (VMamba SS2D).

Strategy: 8 NeuronCores, one (batch, direction) pair per core — B=2 x 4
directions. Each core runs an identical S6 selective-scan program over its
own pre-permuted (L, C) sequence; the host un-permutes and averages the four
directional outputs per batch.

Device layout: channels on partitions (two halves of 128), time on the free
dimension in chunks of T. The recurrence h_t = dA_t*h + dBx_t runs on the
native DVE tensor_tensor_scan (fp32 internal state), dA = exp(A*delta) on the
ACT engine with per-partition scale, softplus via exp+ln (one ACT table), and
the n-contraction y = sum_n C_n*h_n as a strided tensor_reduce.
"""

import sys

try:
    import concourse.bass as bass  # noqa: F401
except ImportError:
    sys.path.insert(0, "/opt/trn_rl_repo")

import numpy as np
import ml_dtypes
import concourse.mybir as mybir
import concourse.bacc as bacc
from concourse import tile
from concourse.bass_utils import run_bass_kernel_spmd

import os

B_, C_, H_, W_, N_ = 2, 256, 64, 64, 16
L_ = H_ * W_
T_ = int(os.environ.get("SS_T", 512))    # time-chunk width (free dim)
NCHUNK = L_ // T_
NCORES = 8
F32 = mybir.dt.float32
BF16 = mybir.dt.bfloat16
DT_LIN = BF16             # dtype of the linear factors (u, B, C, dBx, h, y')
AF = mybir.ActivationFunctionType
OP = mybir.AluOpType

_prog_cache = {}

NCHUNK_BUILD = int(os.environ.get("SS_NCHUNK", NCHUNK))
BC_VIA_DMA = os.environ.get("SS_BC", "dma") == "dma"
INPLACE_LN = os.environ.get("SS_LN", "sep") == "inplace"
REDUCE_ENG = os.environ.get("SS_RED", "pool")


def _build_program():
    if "nc" in _prog_cache:
        return _prog_cache["nc"]
    nc = bacc.Bacc("TRN2", target_bir_lowering=False, debug=False,
                   num_devices=NCORES)
    xt_d = nc.dram_tensor("xt", [C_, L_], F32, kind="ExternalInput")
    wd_d = nc.dram_tensor("wd", [C_, C_], F32, kind="ExternalInput")
    brow_d = nc.dram_tensor("brow", [N_, L_], DT_LIN, kind="ExternalInput")
    crow_d = nc.dram_tensor("crow", [N_, L_], DT_LIN, kind="ExternalInput")
    asc_d = nc.dram_tensor("asc", [128, 2 * N_], F32, kind="ExternalInput")
    bsc_d = nc.dram_tensor("bsc", [128, 2], F32, kind="ExternalInput")
    yt_d = nc.dram_tensor("yt", [C_, L_], F32, kind="ExternalOutput")

    with tile.TileContext(nc) as tc:
        with (
            tc.tile_pool(name="const", bufs=1) as cpool,
            tc.tile_pool(name="work", bufs=2) as wpool,
            tc.tile_pool(name="big", bufs=2) as bigpool,
            tc.tile_pool(name="big1", bufs=1) as big1pool,
            tc.tile_pool(name="da", bufs=4) as dapool,
            tc.tile_pool(name="one", bufs=1) as onepool,
            tc.tile_pool(name="psum", bufs=3, space="PSUM") as pspool,
        ):
            wd0 = cpool.tile([128, C_], F32, tag="wd0")
            nc.sync.dma_start(wd0[:], wd_d[0:128, :])
            wd1 = cpool.tile([128, C_], F32, tag="wd1")
            nc.sync.dma_start(wd1[:], wd_d[128:256, :])
            asc = cpool.tile([128, 2 * N_], F32, tag="asc")
            nc.sync.dma_start(asc[:], asc_d[:])
            bsc = cpool.tile([128, 2], F32, tag="bsc")
            nc.sync.dma_start(bsc[:], bsc_d[:])
            state = cpool.tile([128, 2 * N_], F32, tag="state")

            for k in range(NCHUNK_BUILD):
                sl = slice(k * T_, (k + 1) * T_)
                xts = []
                for h in range(2):
                    xt_h = wpool.tile([128, T_], F32, tag=f"xt{h}")
                    nc.sync.dma_start(xt_h[:], xt_d[h * 128:(h + 1) * 128, sl])
                    xts.append(xt_h)
                # Broadcast B/C time-rows to all 128 partitions straight from
                # DRAM (step-0 partition AP on the DMA source).
                bbc = bigpool.tile([128, N_, T_], DT_LIN, tag="bbc")
                cbc = bigpool.tile([128, N_, T_], DT_LIN, tag="cbc")
                if BC_VIA_DMA:
                    if k == 0:
                        # split the first broadcast so chunk 0's dBx (and the
                        # first scans) can start after half the transfer
                        nc.sync.dma_start(
                            bbc[:, 0:N_ // 2, :],
                            brow_d[0:N_ // 2, sl].unsqueeze(0)
                            .broadcast_to([128, N_ // 2, T_]))
                        nc.sync.dma_start(
                            bbc[:, N_ // 2:, :],
                            brow_d[N_ // 2:, sl].unsqueeze(0)
                            .broadcast_to([128, N_ // 2, T_]))
                    else:
                        nc.sync.dma_start(
                            bbc[:], brow_d[:, sl].unsqueeze(0)
                            .broadcast_to([128, N_, T_]))
                    nc.sync.dma_start(
                        cbc[:], crow_d[:, sl].unsqueeze(0).broadcast_to([128, N_, T_])
                    )
                else:
                    brr = wpool.tile([1, N_, T_], DT_LIN, tag="brr")
                    nc.sync.dma_start(brr[0:1, :, :], brow_d[:, sl])
                    nc.gpsimd.partition_broadcast(
                        bbc[:].rearrange("p n t -> p (n t)"),
                        brr[0:1].rearrange("p n t -> p (n t)"))
                    crr = wpool.tile([1, N_, T_], DT_LIN, tag="crr")
                    nc.sync.dma_start(crr[0:1, :, :], crow_d[:, sl])
                    nc.gpsimd.partition_broadcast(
                        cbc[:].rearrange("p n t -> p (n t)"),
                        crr[0:1].rearrange("p n t -> p (n t)"))
                for h in range(2):
                    psd = pspool.tile([128, T_], F32, tag="psd")
                    nc.tensor.matmul(psd[:], wd0[:, h * 128:(h + 1) * 128],
                                     xts[0][:], start=True, stop=False)
                    nc.tensor.matmul(psd[:], wd1[:, h * 128:(h + 1) * 128],
                                     xts[1][:], start=False, stop=True)
                    # softplus(z + b) = ln(1 + exp(z + b)); Exp and Ln live in
                    # the same ACT table so no table reloads.
                    delta = wpool.tile([128, T_], F32, tag=f"delta{h}")
                    if INPLACE_LN:
                        nc.scalar.activation(delta[:], psd[:], AF.Exp,
                                             bias=bsc[:, h:h + 1])
                        nc.scalar.activation(delta[:], delta[:], AF.Ln, bias=1.0)
                    else:
                        esb = onepool.tile([128, T_], F32, tag="esb")
                        nc.scalar.activation(esb[:], psd[:], AF.Exp,
                                             bias=bsc[:, h:h + 1])
                        nc.scalar.activation(delta[:], esb[:], AF.Ln, bias=1.0)
                    u_h = wpool.tile([128, T_], DT_LIN, tag=f"u{h}")
                    nc.gpsimd.tensor_tensor(out=u_h[:], in0=delta[:],
                                            in1=xts[h][:], op=OP.mult)
                    dbx = big1pool.tile([128, N_, T_], DT_LIN, tag=f"dbx{h}")
                    if k == 0:
                        for half in range(2):
                            ns = slice(half * N_ // 2, (half + 1) * N_ // 2)
                            nc.vector.tensor_tensor(
                                out=dbx[:, ns, :],
                                in0=u_h[:].unsqueeze(1)
                                .broadcast_to([128, N_ // 2, T_]),
                                in1=bbc[:, ns, :], op=OP.mult)
                    else:
                        nc.vector.tensor_tensor(
                            out=dbx[:],
                            in0=u_h[:].unsqueeze(1).broadcast_to([128, N_, T_]),
                            in1=bbc[:], op=OP.mult)
                    hbig = bigpool.tile([128, N_, T_], DT_LIN, tag=f"h{h}")
                    for n in range(N_):
                        idx = h * N_ + n
                        da = dapool.tile([128, T_], F32, tag="da")
                        nc.scalar.activation(da[:], delta[:], AF.Exp,
                                             scale=asc[:, idx:idx + 1])
                        init = 0.0 if k == 0 else state[:, idx:idx + 1]
                        nc.vector.tensor_tensor_scan(
                            out=hbig[:, n, :], data0=da[:], data1=dbx[:, n, :],
                            initial=init, op0=OP.mult, op1=OP.add)
                        nc.scalar.copy(state[:, idx:idx + 1],
                                       hbig[:, n, T_ - 1:T_])
                    yp = big1pool.tile([128, N_, T_], DT_LIN, tag=f"dbx{h}")
                    nc.vector.tensor_tensor(out=yp[:], in0=hbig[:], in1=cbc[:],
                                            op=OP.mult)
                    if REDUCE_ENG == "pool":
                        # n-contraction as a pairwise tree on GPSIMD (DVE is
                        # the critical path; Pool is otherwise idle).
                        q = bigpool.tile([128, N_ // 2, T_], F32, tag=f"h{h}")
                        nc.gpsimd.tensor_tensor(out=q[:], in0=yp[:, 0:N_:2, :],
                                                in1=yp[:, 1:N_:2, :], op=OP.add)
                        m = N_ // 2
                        while m > 1:
                            nc.gpsimd.tensor_tensor(
                                out=q[:, 0:m // 2, :], in0=q[:, 0:m // 2, :],
                                in1=q[:, m // 2:m, :], op=OP.add)
                            m //= 2
                        yv = q[:, 0, :]
                    else:
                        yvt = wpool.tile([128, T_], F32, tag=f"yv{h}")
                        nc.vector.tensor_reduce(out=yvt[:],
                                                in_=yp[:].transpose([0, 2, 1]),
                                                axis=mybir.AxisListType.X,
                                                op=OP.add)
                        yv = yvt[:]
                    # the x*D skip term is added on the host
                    nc.sync.dma_start(yt_d[h * 128:(h + 1) * 128, sl], yv)

    # All ACT funcs used (Exp, Ln, Copy) live in one table; restrict the
    # table list so insert_act_table_loads emits a single load instead of
    # ping-ponging between the exp-only and ln-only tables (1.3us each).
    import concourse.hw_specs as hw_specs
    orig_tables = hw_specs.get_activation_tables
    def _one_table(arch):
        # Keep every table at its original index (the emitted act_func_set_id
        # is positional), but strip Exp/Ln/Copy from all tables except
        # natural_log_exp_and_others so the chooser settles on that one.
        tabs = orig_tables(arch)
        keep = {AF.Exp, AF.Ln, AF.Copy}
        out = {}
        for name, funcs in tabs.items():
            if name == "natural_log_exp_and_others":
                out[name] = funcs
            else:
                out[name] = {f for f in funcs if f not in keep}
        return out
    hw_specs.get_activation_tables = _one_table
    try:
        import concourse.bacc as _bacc_mod
        _bacc_saved = _bacc_mod.get_activation_tables
        _bacc_mod.get_activation_tables = _one_table
        nc.compile()
    finally:
        hw_specs.get_activation_tables = orig_tables
        _bacc_mod.get_activation_tables = _bacc_saved
    _prog_cache["nc"] = nc
    return nc


def _permute_inputs(features, b, d):
    """Return the direction-d scan sequence of batch b as an (L, C) array."""
    f = np.asarray(features[b])                      # (C, H, W)
    if d == 0:
        return f.reshape(C_, L_).T
    if d == 1:
        return f.reshape(C_, L_).T[::-1]
    if d == 2:
        return f.transpose(1, 2, 0)[::-1].reshape(L_, C_)
    return f.transpose(2, 1, 0)[::-1].reshape(L_, C_)


def _unpermute_output(y, d):
    """Map a direction-d scan output (L, C) back to row-major (L, C)."""
    if d == 0:
        return y
    if d == 1:
        return y[::-1]
    if d == 2:
        return y.reshape(H_, W_, C_)[::-1].reshape(L_, C_)
    return y.reshape(W_, H_, C_)[::-1].transpose(1, 0, 2).reshape(L_, C_)


def kernel(features, A_log, D, W_delta, b_delta, W_B, W_C):
    features = np.asarray(features, np.float32)
    A_log = np.asarray(A_log, np.float32)
    D = np.asarray(D, np.float32)
    W_delta = np.asarray(W_delta, np.float32)
    b_delta = np.asarray(b_delta, np.float32)
    W_B = np.asarray(W_B, np.float32)
    W_C = np.asarray(W_C, np.float32)

    A = -np.exp(A_log)                                        # (C, N)
    asc = np.ascontiguousarray(np.concatenate([A[:128], A[128:]], axis=1))
    bsc = np.ascontiguousarray(b_delta.reshape(2, 128).T)

    in_maps = []
    for core in range(NCORES):
        b, d = divmod(core, 4)
        x = _permute_inputs(features, b, d)                   # (L, C)
        xt = np.ascontiguousarray(x.T, np.float32)
        brow = np.ascontiguousarray((x @ W_B).T).astype(ml_dtypes.bfloat16)
        crow = np.ascontiguousarray((x @ W_C).T).astype(ml_dtypes.bfloat16)
        in_maps.append({
            "xt": xt, "wd": W_delta,
            "brow": brow, "crow": crow, "asc": asc, "bsc": bsc,
        })

    nc = _build_program()
    res = run_bass_kernel_spmd(nc, in_maps, list(range(NCORES)))

    out = np.zeros((B_, L_, C_), np.float32)
    for core in range(NCORES):
        b, d = divmod(core, 4)
        x = _permute_inputs(features, b, d)
        y = res.results[core]["yt"].T + x * D                 # (L, C)
        out[b] += _unpermute_output(y, d)
    out /= 4.0
    return np.ascontiguousarray(out.transpose(0, 2, 1).reshape(B_, C_, H_, W_))



# revision 2
# speedup vs baseline: 1.0322x; 1.0322x over previous
"""Trainium2 Bass kernel v2 for the 4-direction cross selective scan.

Layout G: partitions = (8 channels x 16 states) per group, 32 groups/core,
time on the free dim. One (batch, direction) pair per core (8 cores).

Key structure (per core):
 - PE: delta matmuls (f32r), m = a*delta + ln(rho) cube via per-group [16,128]
   stationaries (delta rows + lnrho rows stacked in the contraction dim),
   y-reduce over n via 0/1 stationaries accumulating in PSUM.
 - ACT: softplus, then one exp per (group, Tq) straight PSUM -> PSUM.
 - DVE/Pool: scans h~ = scan(dA', xB) with data1 = host-provided x*B
   (delta folded out of data1 by the substitution h~ = h/delta, which turns
   data0 into dA' = exp(a*delta + ln(delta_{t-1}/delta_t))), yp = h~ * Crep
   at DVE 2x, final y = delta * psum(y-reduce).
Host adds the x*D skip term and averages the 4 directions.
"""

import sys

try:
    import concourse.bass as bass  # noqa: F401
except ImportError:
    sys.path.insert(0, "/opt/trn_rl_repo")

import os
import numpy as np
import ml_dtypes
import concourse.mybir as mybir
import concourse.bacc as bacc
from concourse import tile
from concourse.bass_utils import run_bass_kernel_spmd

B_, C_, H_, W_, N_ = 2, 256, 64, 64, 16
L_ = H_ * W_            # 4096
G_ = 32                 # groups (8 channels x 16 states each)
GH = 16                 # groups per half
TQ = 1024               # time quarter
NTQ = L_ // TQ          # 4
NCORES = 8
F32 = mybir.dt.float32
F32R = mybir.dt.float32r
BF16 = mybir.dt.bfloat16
AF = mybir.ActivationFunctionType
OP = mybir.AluOpType

# scans on DVE for local group index j < DVE_SCANS, else Pool
DVE_SCANS = int(os.environ.get("SS_DVESCANS", 5))

_prog_cache = {}


def _build_program():
    if "nc" in _prog_cache:
        return _prog_cache["nc"]
    nc = bacc.Bacc("TRN2", target_bir_lowering=False, debug=False,
                   num_devices=NCORES)
    xt_d = nc.dram_tensor("xt", [C_, L_], BF16, kind="ExternalInput")
    wd_d = nc.dram_tensor("wd", [C_, C_], BF16, kind="ExternalInput")
    bsc_d = nc.dram_tensor("bsc", [128, 2], F32, kind="ExternalInput")
    as2_d = nc.dram_tensor("as2", [128, G_ * 128], BF16, kind="ExternalInput")
    sy_d = nc.dram_tensor("sy", [128, GH * 128], BF16, kind="ExternalInput")
    crep_d = nc.dram_tensor("crep", [128, L_], BF16, kind="ExternalInput")
    # xb laid out [Tq, half, p, j*t] so each (Tq, half) is one contiguous DMA
    # with per-partition-contiguous 32KB runs
    xb_d = nc.dram_tensor("xb", [NTQ, 2, 128, GH * TQ], BF16,
                          kind="ExternalInput")
    yt_d = nc.dram_tensor("yt", [C_, L_], F32, kind="ExternalOutput")

    with tile.TileContext(nc) as tc:
        with (
            tc.tile_pool(name="const", bufs=1) as cpool,
            tc.tile_pool(name="xb", bufs=3) as xbpool,
            tc.tile_pool(name="work", bufs=3) as wpool,
            tc.tile_pool(name="yout", bufs=2) as ypool,
            tc.tile_pool(name="psm", bufs=3, space="PSUM") as psmpool,
            tc.tile_pool(name="psy", bufs=1, space="PSUM") as psypool,
        ):
            wd0 = cpool.tile([128, C_], BF16, tag="wd0")
            nc.sync.dma_start(wd0[:], wd_d[0:128, :])
            wd1 = cpool.tile([128, C_], BF16, tag="wd1")
            nc.sync.dma_start(wd1[:], wd_d[128:256, :])
            bsc = cpool.tile([128, 2], F32, tag="bsc")
            nc.sync.dma_start(bsc[:], bsc_d[:])
            # phase-2 constants allocated here but DMA'd after the phase-1
            # x loads are in flight (they're not needed until phase 2)
            as2 = cpool.tile([128, G_ * 128], BF16, tag="as2")
            sy = cpool.tile([128, GH * 128], BF16, tag="sy")
            crep = cpool.tile([128, L_], BF16, tag="crep")
            states = cpool.tile([128, G_], F32, tag="states")
            zcol = cpool.tile([128, 1], F32, tag="zcol")
            nc.gpsimd.memset(zcol[:], 0.0)

            deltas, lnrs, dls = [], [], []
            for h in range(2):
                deltas.append(cpool.tile([128, L_], BF16, tag=f"delta{h}",
                                         name=f"delta{h}"))
                lnrs.append(cpool.tile([128, L_], BF16, tag=f"lnr{h}",
                                       name=f"lnr{h}"))
                dls.append([cpool.tile([128, L_], BF16, tag=f"dl{h}{s}",
                                       name=f"dl{h}{s}") for s in range(2)])

            # xb streaming: one block (q, h) ahead, two 8-group sub-tiles per
            # block, rolling through 3 buffers. All half-0 blocks run first;
            # half-1's phase-1 work is interleaved into their ACT slack.
            blocks = [(q, 0) for q in range(NTQ)] + [(q, 1) for q in range(NTQ)]
            xb_tiles = {}

            def issue_xb(i, b2):
                if i >= len(blocks):
                    return
                bq, bh = blocks[i]
                t = xbpool.tile([128, 8 * TQ], BF16, tag="xb", name="xbt")
                nc.sync.dma_start(
                    t[:], xb_d[bq, bh, :, b2 * 8 * TQ:(b2 + 1) * 8 * TQ])
                xb_tiles[(i, b2)] = t

            # ---- phase 1 emitters: softplus chunks, then per-half tail
            # (ln(delta), lnrho, dl stacking)
            xpool = cpool
            xfs = []

            def ph1_chunk(h, k):
                sl = slice(k * TQ, (k + 1) * TQ)
                psd = psmpool.tile([128, TQ], F32, tag="psm", name="psd")
                for s in range(2):
                    psl = slice(k * TQ + s * 512, k * TQ + (s + 1) * 512)
                    opsl = slice(s * 512, (s + 1) * 512)
                    nc.tensor.matmul(
                        psd[:, opsl], wd0[:, h * 128:(h + 1) * 128],
                        xfs[0][:, psl], start=True, stop=False)
                    nc.tensor.matmul(
                        psd[:, opsl], wd1[:, h * 128:(h + 1) * 128],
                        xfs[1][:, psl], start=False, stop=True)
                esb = ypool.tile([128, TQ], BF16, tag="esb", name="esb")
                nc.scalar.activation(esb[:], psd[:], AF.Exp,
                                     bias=bsc[:, h:h + 1])
                nc.scalar.activation(deltas[h][:, sl], esb[:], AF.Ln,
                                     bias=1.0)

            def ph1_lnr_chunk(h, k, lnp):
                # lnrho_t = ln(delta_{t-1}) - ln(delta_t) for chunk k, with a
                # 1-col overlap read into chunk k-1 (delta_{-1} := 1)
                sl = slice(k * TQ, (k + 1) * TQ)
                if k == 0:
                    nc.vector.memset(lnp[:, 0:1], 0.0)
                    nc.scalar.activation(lnp[:, 1:TQ + 1], deltas[h][:, sl],
                                         AF.Ln)
                else:
                    nc.scalar.activation(
                        lnp[:], deltas[h][:, k * TQ - 1:(k + 1) * TQ], AF.Ln)
                nc.vector.tensor_tensor(
                    out=lnrs[h][:, sl], in0=lnp[:, 0:TQ],
                    in1=lnp[:, 1:TQ + 1], op=OP.subtract)
                # stack this chunk's delta/lnrho rows into dl immediately
                # (partitions 0..63 delta, 64..127 lnrho, per 8-group sub):
                # chunk k of dl is all block (q=k, h) needs, so phase 2 can
                # start as soon as the first chunk lands
                for s in range(2):
                    src = slice(64 * s, 64 * (s + 1))
                    nc.sync.dma_start(dls[h][s][0:64, sl],
                                      deltas[h][src, sl])
                    nc.sync.dma_start(dls[h][s][64:128, sl],
                                      lnrs[h][src, sl])

            def new_lnp():
                return xpool.tile([128, TQ + 1], BF16, tag="lnp", name="lnp",
                                  bufs=2)

            def ph1_tail(h):
                for k in range(NTQ):
                    ph1_lnr_chunk(h, k, new_lnp())

            # prologue DMAs: x first (phase 1 starts on it), then phase-2
            # prefetches in need order
            for kh in range(2):
                x_f = xpool.tile([128, L_], BF16, tag=f"x{kh}",
                                 name=f"x{kh}", bufs=1)
                nc.sync.dma_start(x_f[:], xt_d[kh * 128:(kh + 1) * 128, :])
                xfs.append(x_f)
            nc.sync.dma_start(as2[:], as2_d[:])
            issue_xb(0, 0)
            nc.sync.dma_start(crep[:], crep_d[:])
            nc.sync.dma_start(sy[:], sy_d[:])
            # half 0: lnrho chunks interleave right behind the softplus
            # chunks so the dl stack can fire immediately after chunk 3
            for k in range(NTQ):
                ph1_chunk(0, k)
                ph1_lnr_chunk(0, k, new_lnp())
            issue_xb(0, 1)
            issue_xb(1, 0)
            # half-1 phase 1 is emitted inside the phase-2 pipeline (hooks
            # below) to fill ACT slack during the half-0 blocks


            # ---- phase 2: flat software pipeline over (block, group) with a
            # LAG-group y-reduce so PE never head-blocks across block edges
            LAG = 4
            # groups whose yp mult runs on DVE (rest on Pool)
            DVE_SET = {0, 3, 6, 9, 12, 15}
            nitems = len(blocks) * GH
            psys = {}
            ypss = {}

            def y_reduce(idx):
                bi2, j2 = divmod(idx, GH)
                for s in range(2):
                    psl = slice(s * 512, (s + 1) * 512)
                    nc.tensor.matmul(
                        psys[bi2][:, psl],
                        sy[:, j2 * 128:(j2 + 1) * 128],
                        ypss[idx][:, psl],
                        start=(j2 == 0), stop=(j2 == GH - 1))
                if j2 == GH - 1:
                    bq, bh = blocks[bi2]
                    bsl = slice(bq * TQ, (bq + 1) * TQ)
                    # psy is PSUM: GPSIMD can't touch it, so this stays on DVE
                    yo = ypool.tile([128, TQ], F32, tag="yo", name="yo")
                    nc.vector.tensor_tensor(out=yo[:], in0=psys[bi2][:],
                                            in1=deltas[bh][:, bsl],
                                            op=OP.mult)
                    nc.scalar.dma_start(
                        yt_d[bh * 128:(bh + 1) * 128, bsl], yo[:])
                    del psys[bi2]
                del ypss[idx]

            for idx in range(nitems + LAG):
                if idx < nitems:
                    bi, j = divmod(idx, GH)
                    q, h = blocks[bi]
                    sl = slice(q * TQ, (q + 1) * TQ)
                    g = h * GH + j
                    if j == 0:
                        psys[bi] = psypool.tile([128, TQ], F32, tag="psy",
                                                name="psy")
                    psm = psmpool.tile([128, TQ], F32, tag="psm", name="psm")
                    dl = dls[h][j // 8]
                    for s in range(2):
                        ssl = slice(q * TQ + s * 512, q * TQ + (s + 1) * 512)
                        psl = slice(s * 512, (s + 1) * 512)
                        nc.tensor.matmul(
                            psm[:, psl],
                            as2[:, g * 128:(g + 1) * 128],
                            dl[:, ssl],
                            start=True, stop=True)
                    da = wpool.tile([128, TQ], BF16, tag="da", name="da",
                                    bufs=6)
                    nc.scalar.activation(da[:], psm[:], AF.Exp)
                    ht = wpool.tile([128, TQ], BF16, tag="ht", name="ht",
                                    bufs=4)
                    init = 0.0 if q == 0 else states[:, g:g + 1]
                    # scans are DVE-only (TensorScalarPtr is illegal on the
                    # Pool engine in the real ISA); Pool absorbs most of the
                    # yp mults, the state copies, and the final y-mult
                    nc.vector.tensor_tensor_scan(
                        out=ht[:], data0=da[:],
                        data1=xb_tiles[(bi, j // 8)][
                            :, (j % 8) * TQ:(j % 8 + 1) * TQ],
                        initial=init, op0=OP.mult, op1=OP.add)
                    if q < NTQ - 1:
                        nc.gpsimd.tensor_tensor(
                            out=states[:, g:g + 1], in0=ht[:, TQ - 1:TQ],
                            in1=zcol[:], op=OP.add)
                    yp = wpool.tile([128, TQ], BF16, tag="yp", name="yp",
                                    bufs=6)
                    ypeng = nc.vector if j in DVE_SET else nc.gpsimd
                    ypeng.tensor_tensor(out=yp[:], in0=ht[:],
                                        in1=crep[:, sl], op=OP.mult)
                    ypss[idx] = yp
                    if j == 7:
                        issue_xb(bi + 1, 1)
                    elif j == GH - 1:
                        issue_xb(bi + 2, 0)
                    # half-1 phase-1 emission hooks: fill ACT slack of the
                    # first half-0 blocks
                    if (bi, j) in ((0, 6), (0, 12), (1, 6), (1, 12)):
                        ph1_chunk(1, (2 * bi + (j == 12)) % NTQ)
                    elif (bi, j) == (2, 2):
                        ph1_tail(1)
                if idx >= LAG:
                    y_reduce(idx - LAG)

    # Single ACT table (Exp + Ln) to avoid table reloads.
    import concourse.hw_specs as hw_specs
    orig_tables = hw_specs.get_activation_tables

    def _one_table(arch):
        tabs = orig_tables(arch)
        keep = {AF.Exp, AF.Ln, AF.Copy}
        out = {}
        for name, funcs in tabs.items():
            if name == "natural_log_exp_and_others":
                out[name] = funcs
            else:
                out[name] = {f for f in funcs if f not in keep}
        return out

    hw_specs.get_activation_tables = _one_table
    import concourse.bacc as _bacc_mod
    _bacc_saved = _bacc_mod.get_activation_tables
    _bacc_mod.get_activation_tables = _one_table
    try:
        nc.compile()
    finally:
        hw_specs.get_activation_tables = orig_tables
        _bacc_mod.get_activation_tables = _bacc_saved
    _prog_cache["nc"] = nc
    return nc


def _permute_inputs(features, b, d):
    f = np.asarray(features[b])                      # (C, H, W)
    if d == 0:
        return f.reshape(C_, L_).T
    if d == 1:
        return f.reshape(C_, L_).T[::-1]
    if d == 2:
        return f.transpose(1, 2, 0)[::-1].reshape(L_, C_)
    return f.transpose(2, 1, 0)[::-1].reshape(L_, C_)


def _unpermute_output(y, d):
    if d == 0:
        return y
    if d == 1:
        return y[::-1]
    if d == 2:
        return y.reshape(H_, W_, C_)[::-1].reshape(L_, C_)
    return y.reshape(W_, H_, C_)[::-1].transpose(1, 0, 2).reshape(L_, C_)


def _make_consts(A_log, b_delta, W_B):
    """Core-independent constant tensors: as2, sy, bsc."""
    A = -np.exp(np.asarray(A_log, np.float32))            # (C, N)
    as2 = np.zeros((128, G_ * 128), np.float32)
    for g in range(G_):
        blk = as2[:, g * 128:(g + 1) * 128]
        jj = (g % 16) % 8
        for cb in range(8):
            c = 8 * g + cb
            blk[8 * jj + cb, 16 * cb:16 * cb + 16] = A[c]
            blk[64 + 8 * jj + cb, 16 * cb:16 * cb + 16] = 1.0
    sy = np.zeros((128, GH * 128), np.float32)
    for j in range(GH):
        for cb in range(8):
            for n in range(N_):
                sy[16 * cb + n, j * 128 + 8 * j + cb] = 1.0
    bsc = np.ascontiguousarray(np.asarray(b_delta, np.float32).reshape(2, 128).T)
    return (as2.astype(ml_dtypes.bfloat16), sy.astype(ml_dtypes.bfloat16), bsc)


def _make_xb(x, W_B):
    """xb [Tq, half, j, 128, TQ] bf16 with xb[..., 16*cb+n, t] = x[t,c]*Bm[t,n]."""
    Bm = (x @ W_B).astype(np.float32)                     # (L, N)
    xT = np.ascontiguousarray(x.T, np.float32)            # (C, L)
    xb = np.empty((NTQ, 2, 128, GH, TQ), np.float32)
    BmT = np.ascontiguousarray(Bm.T)                      # (N, L)
    for q in range(NTQ):
        tsl = slice(q * TQ, (q + 1) * TQ)
        Bq = BmT[:, tsl]                                  # (N, TQ)
        for h in range(2):
            for j in range(GH):
                c0 = 128 * h + 8 * j
                blk = xT[c0:c0 + 8, tsl]                  # (8, TQ)
                # partition p = 16*cb + n
                xb[q, h, :, j, :] = (
                    blk[:, None, :] * Bq[None, :, :]).reshape(128, TQ)
    return xb.reshape(NTQ, 2, 128, GH * TQ).astype(ml_dtypes.bfloat16)


def _make_crep(x, W_C):
    Cm = (x @ W_C).astype(np.float32)                     # (L, N)
    CmT = np.ascontiguousarray(Cm.T)                      # (N, L)
    return np.tile(CmT, (8, 1)).astype(ml_dtypes.bfloat16)  # (128, L)


def kernel(features, A_log, D, W_delta, b_delta, W_B, W_C):
    features = np.asarray(features, np.float32)
    A_log = np.asarray(A_log, np.float32)
    D = np.asarray(D, np.float32)
    W_delta = np.asarray(W_delta, np.float32)
    b_delta = np.asarray(b_delta, np.float32)
    W_B = np.asarray(W_B, np.float32)
    W_C = np.asarray(W_C, np.float32)

    as2, sy, bsc = _make_consts(A_log, b_delta, W_B)

    in_maps = []
    for core in range(NCORES):
        b, d = divmod(core, 4)
        x = _permute_inputs(features, b, d)               # (L, C)
        in_maps.append({
            "xt": np.ascontiguousarray(x.T).astype(ml_dtypes.bfloat16),
            "wd": W_delta.astype(ml_dtypes.bfloat16),
            "bsc": bsc, "as2": as2, "sy": sy,
            "crep": _make_crep(x, W_C),
            "xb": _make_xb(x, W_B),
        })

    nc = _build_program()
    res = run_bass_kernel_spmd(nc, in_maps, list(range(NCORES)))

    out = np.zeros((B_, L_, C_), np.float32)
    for core in range(NCORES):
        b, d = divmod(core, 4)
        x = _permute_inputs(features, b, d)
        y = res.results[core]["yt"].T + x * D             # (L, C)
        out[b] += _unpermute_output(y, d)
    out /= 4.0
    return np.ascontiguousarray(out.transpose(0, 2, 1).reshape(B_, C_, H_, W_))
